# revision 48
# baseline (speedup 1.0000x reference)
"""Distributed bidirectional ChebConv (K=5) kernel for Trainium2 (8 NeuronCores).

Self-contained: kernel(**inputs) takes the FULL inputs, shards internally,
compiles + runs a Bass/Tile SPMD program on cores 0-7 via
concourse.bass_utils.run_bass_kernel_spmd, and returns the FULL [N, 128] output.

Strategy (graph/data parallel):
 - Per conv, nodes are globally degree-sorted (by edge count) and snake-dealt
   across the 8 cores: each core owns ~equal total edges and its 128-node tiles
   are degree-homogeneous (so padded-CSR padding is ~2%).
 - Each core keeps its Chebyshev state T_k node-major in SBUF.  Per step,
   U = D^-1/2 * T is AllGathered (DRAM, Shared outputs) into a replicated
   table; each core dma_gathers U[col] for its padded-CSR edge slots as
   single 256B fp32 node rows.  The int16 gather indices are SIGNED, biased
   by -TBL/2 with the table AP sliced to its midpoint (the Q7 descgen address
   multiply is unsigned*signed), which covers all 50176 padded nodes with no
   pair-fetch waste.  Each gather group carries one trailing sentinel slot
   (idx 0, weight 0) so the ucode's trailing-negative-index trim never drops
   a real slot.  Weights multiply + per-tile tensor_reduce run on DVE.
 - The gather descriptor generation on the Pool engine's Q7 cores is the
   kernel bottleneck: gathers round-robin over 4 SWDGE queues (each queue is
   served by a different Q7 core pair), so consecutive instructions' descgen
   overlaps via sequencer run-ahead (~2x).
 - U0 = D^-1/2 x is host-precomputed and uploaded as a full replicated table,
   eliminating the k=1 AllGather from the critical path.
 - Chebyshev recurrence, the per-k matmul with W (PE transpose + matmul) and
   the bias add run on-core; host reassembles/unpermutes the shards.
"""
import numpy as np
from contextlib import ExitStack

from concourse import bacc, tile, mybir, bass_utils
from concourse.masks import make_identity

F32 = mybir.dt.float32
F16 = mybir.dt.float16
I16 = mybir.dt.int16
P = 128
NCORES = 8
K = 5
SCALE = 2.0 / 3.0  # 2 / lambda_max with lambda_max = 3.0

TRACE = False          # test harness sets True to collect an NTFF profile
LAST_EXEC_NS = None
LAST_RESULTS = None


# ----------------------------------------------------------------- host prep
def _prep_conv(row, col, ew, N, padt, sgmax):
    PADN = padt * P
    BIAS = NCORES * PADN // 2
    E = len(row)
    cnt = np.bincount(row, minlength=N)
    order = np.argsort(-cnt, kind="stable")  # position -> orig node id
    npos = np.arange(N)
    blk = npos // NCORES
    j = npos % NCORES
    core_of_pos = np.where(blk % 2 == 0, j, NCORES - 1 - j)
    newid_of_pos = core_of_pos * PADN + blk
    iperm = np.empty(N, np.int64)
    iperm[order] = newid_of_pos  # orig node -> padded global newid

    nrow = iperm[row]
    ncol = iperm[col]
    ecore = nrow // PADN
    rloc = nrow % PADN
    sidx = np.argsort(nrow, kind="stable")
    srow = nrow[sidx]
    first = np.searchsorted(srow, srow, side="left")
    d = np.empty(E, np.int64)
    d[sidx] = np.arange(E) - first
    t_of_e = rloc // P
    p_of_e = rloc % P

    Dt = np.zeros(padt, np.int64)
    np.maximum.at(Dt, t_of_e, d + 1)
    Dt = np.maximum(Dt, 1)
    sgmax = max(sgmax, int(Dt.max()))

    # Pack tiles into gather groups of <= sgmax data slots; each group gets one
    # trailing sentinel slot (idx 0, weight 0) so the ucode's trailing-negative
    # index trim can never drop a real (possibly negative, signed-biased) idx.
    groups = []
    t0, acc = 0, 0
    for t in range(padt):
        if acc + Dt[t] > sgmax and acc > 0:
            groups.append((t0, t, acc + 1))
            t0, acc = t, 0
        acc += Dt[t]
    groups.append((t0, padt, acc + 1))

    slot_base = np.zeros(padt, np.int64)
    base = 0
    for (ta, tb, sg) in groups:
        o = base
        for t in range(ta, tb):
            slot_base[t] = o
            o += Dt[t]
        base += sg
    stot = base

    slot_flat = slot_base[t_of_e] + d
    nodeidx = (ncol - BIAS).astype(np.int16)  # signed single-node table index

    w2 = np.zeros((NCORES, P, stot), np.float32)
    w2[ecore, p_of_e, slot_flat] = ew
    idxflat = np.zeros((NCORES, stot * P), np.int16)
    idxflat[ecore, slot_flat * P + p_of_e] = nodeidx
    # SBUF layout: list position i -> [i % 16, i // 16], replicated 8x over
    # partition groups of 16 (one copy per Q7 core).
    lay = idxflat.reshape(NCORES, stot * P // 16, 16).transpose(0, 2, 1)
    lay = np.tile(lay, (1, 8, 1))  # [NCORES, 128, stot*8]

    return dict(iperm=iperm, Dt=Dt, groups=groups, slot_base=slot_base,
                stot=stot, sgmax=sgmax, w2=w2, idx=np.ascontiguousarray(lay))


def prep(x, edge_index, edge_weight, sgmax=40):
    N, C = x.shape
    assert N % NCORES == 0
    padt = -(-(N // NCORES) // P)  # ceil
    PADN = padt * P
    row = np.asarray(edge_index[0], np.int64)
    col = np.asarray(edge_index[1], np.int64)
    ew = np.asarray(edge_weight, np.float32)

    def dis_of(r):
        deg = np.zeros(N, np.float64)
        np.add.at(deg, r, ew.astype(np.float64))
        return np.where(deg > 0, 1.0 / np.sqrt(np.maximum(deg, 1e-30)), 0.0)

    # Fold the full off-diagonal weight -scale * dis[row] * ew * dis[col] into
    # the per-edge table: the device recurrence then needs no per-node
    # rescaling and the AllGather table is the raw Chebyshev state.
    dis1, dis2 = dis_of(row), dis_of(col)
    wf1 = (-SCALE * dis1[row] * ew * dis1[col]).astype(np.float32)
    wf2 = (-SCALE * dis2[col] * ew * dis2[row]).astype(np.float32)
    m1 = _prep_conv(row, col, wf1, N, padt, sgmax)
    m2 = _prep_conv(col, row, wf2, N, padt, sgmax)

    def shards(arr, m):
        xs = np.zeros((NCORES * PADN, C), np.float32)
        xs[m["iperm"]] = np.asarray(arr, np.float32)
        return xs.reshape(NCORES, PADN, C)

    x = np.asarray(x, np.float32)
    # Full replicated T0 (= x) tables: uploaded as inputs (free), so the k=1
    # AllGather disappears from the critical path entirely.
    u0f1 = shards(x, m1).reshape(NCORES * PADN, C)
    u0f2 = shards(x, m2).reshape(NCORES * PADN, C)
    return dict(m1=m1, m2=m2, padt=padt, PADN=PADN, N=N, C=C,
                sgmax=max(m1["sgmax"], m2["sgmax"]),
                xs1=shards(x, m1), xs2=shards(x, m2),
                u0f1=u0f1, u0f2=u0f2)


# ------------------------------------------------------------- device build
def build(meta):
    """Conv-interleaved build: per step, conv A's AllGather transfer (CC cores)
    hides under conv B's gather phase (Q7/DMA) and vice versa."""
    padt, C, sgmax = meta["padt"], meta["C"], meta["sgmax"]
    PADN = padt * P
    TBL = NCORES * PADN
    m = [meta["m1"], meta["m2"]]
    nc = bacc.Bacc("TRN2", target_bir_lowering=False, debug=False,
                   enable_asserts=True, num_devices=NCORES,
                   num_swdge_queues=4)

    xs_d = [nc.dram_tensor(f"xs{ci}", [PADN, C], F32, kind="ExternalInput").ap()
            for ci in range(2)]
    u0f_d = [nc.dram_tensor(f"u0f{ci}", [TBL, C], F32, kind="ExternalInput").ap()
             for ci in range(2)]
    idx_d = [nc.dram_tensor(f"idx{ci}", [P, m[ci]["stot"] * 8], I16,
                            kind="ExternalInput").ap() for ci in range(2)]
    w2_d = [nc.dram_tensor(f"w2{ci}", [P, m[ci]["stot"]], F32,
                           kind="ExternalInput").ap() for ci in range(2)]
    wmat_d = nc.dram_tensor("wmat", [C, 2 * K * C], F32, kind="ExternalInput").ap()
    brep_d = nc.dram_tensor("brep", [P, 2 * C], F32, kind="ExternalInput").ap()
    out_d = [nc.dram_tensor(f"out{ci}", [PADN, C], F32, kind="ExternalOutput").ap()
             for ci in range(2)]

    def nm(ap):
        return ap.rearrange("(t p) f -> p t f", p=P)

    def sb3(t):
        return t[:].rearrange("p (t f) -> p t f", f=C)

    with tile.TileContext(nc) as tc, ExitStack() as ctx:
        sb = ctx.enter_context(tc.tile_pool(name="sb", bufs=1))
        gp = ctx.enter_context(tc.tile_pool(name="gp", bufs=6))
        ps = ctx.enter_context(tc.tile_pool(name="ps", bufs=2, space="PSUM"))
        dr = ctx.enter_context(tc.tile_pool(name="dr", bufs=1, space="DRAM"))

        ag_in = [dr.tile([PADN, C], F32, name=f"agin{i}") for i in range(2)]
        ag_outs = [dr.tile([TBL, C], F32, addr_space="Shared", name=f"agout{i}")
                   for i in range(2 * (K - 1))]

        Tbuf = [[sb.tile([P, padt * C], F32, name=f"T{ci}{j}") for j in range(2)]
                for ci in range(2)]
        accs = [sb.tile([P, padt * C], F32, name=f"acc{ci}") for ci in range(2)]
        R = sb.tile([P, padt * C], F32, tag="R")
        Wsb = sb.tile([C, 2 * K * C], F32, tag="Wsb")
        brep = sb.tile([P, 2 * C], F32, tag="brep")
        ident = sb.tile([P, P], F32, tag="ident")
        w2sb = [sb.tile([P, m[ci]["stot"]], F32, name=f"w2sb{ci}")
                for ci in range(2)]
        idxsb = [sb.tile([P, m[ci]["stot"] * 8], I16, name=f"idxsb{ci}")
                 for ci in range(2)]
        make_identity(nc, ident[:])
        nc.sync.dma_start(Wsb[:], wmat_d[:])
        nc.sync.dma_start(brep[:], brep_d[:])

        AG = mybir.AluOpType
        bc3 = lambda ap2, n: ap2.unsqueeze(2).broadcast_to([P, n, C])
        Tv = lambda T: T[:].rearrange("p (t f) -> p t f", f=C)

        def matmul_acc(ci, k, Tcur, first, t0=0, t1=None):
            acc = accs[ci]
            for t in range(t0, padt if t1 is None else t1):
                tp = ps.tile([C, P], F32, tag="tp")
                nc.tensor.transpose(out=tp[:], in_=Tcur[:, t * C:(t + 1) * C],
                                    identity=ident[:])
                tf = gp.tile([C, P], F32, tag="tf")
                nc.scalar.copy(out=tf[:], in_=tp[:])
                mp = ps.tile([P, C], F32, tag="mp")
                nc.tensor.matmul(mp[:], lhsT=tf[:],
                                 rhs=Wsb[:, (ci * K + k) * C:(ci * K + k + 1) * C],
                                 start=True, stop=True)
                a = acc[:, t * C:(t + 1) * C]
                if first:
                    nc.vector.tensor_tensor(out=a, in0=mp[:],
                                            in1=brep[:, ci * C:(ci + 1) * C],
                                            op=AG.add)
                else:
                    nc.vector.tensor_tensor(out=a, in0=a, in1=mp[:], op=AG.add)

        def do_allgather(ci, k):
            ag_out = ag_outs[ci * (K - 1) + (k - 1)]
            nc.gpsimd.collective_compute(
                "AllGather", AG.bypass, replica_groups=[list(range(NCORES))],
                ins=[ag_in[ci].opt()], outs=[ag_out.opt()])

        # Round-robin gathers across the 4 SWDGE queues: each queue's
        # descriptor generation runs on a different Q7 core pair, so up to 4
        # gathers' descgen overlaps (the ucode pins queue q to cores 2q,2q+1).
        qrr = [0]

        state = []
        # -------- prologue per conv: x, tables, dis, U0, first AG
        for ci in range(2):
            mm = m[ci]
            stot, Dt, slot_base = mm["stot"], mm["Dt"], mm["slot_base"]
            T0 = Tbuf[ci][0]
            nc.sync.dma_start(idxsb[ci][:], idx_d[ci][:])
            nc.sync.dma_start(w2sb[ci][:], w2_d[ci][:])
            nc.sync.dma_start(sb3(T0), nm(xs_d[ci]))
            matmul_acc(ci, 0, T0, first=True)
            state.append(dict(prev=T0, cur=Tbuf[ci][1]))

        # -------- interleaved Chebyshev steps
        def chain(ci, k, t0, t1):
            """Sliced recurrence + ag_in staging for tiles [t0, t1).
            Weights are fully dis-folded on the host: R is already the
            off-diagonal term of L_hat applied to T_{k-1}, and the staged
            table is the raw state (no per-node rescaling needed)."""
            a, b = t0 * C, t1 * C
            st = state[ci]
            prev, cur = st["prev"], st["cur"]
            dw = SCALE - 1.0
            Rs = R[:, a:b]
            if k == 1:
                nc.vector.scalar_tensor_tensor(
                    out=cur[:, a:b], in0=prev[:, a:b], scalar=dw, in1=Rs,
                    op0=AG.mult, op1=AG.add)
                newcur = cur
            else:
                # T_k = 2*dw*T_{k-1} + 2*R - T_{k-2}
                nc.vector.scalar_tensor_tensor(
                    out=Rs, in0=Rs, scalar=2.0, in1=prev[:, a:b],
                    op0=AG.mult, op1=AG.subtract)
                nc.vector.scalar_tensor_tensor(
                    out=prev[:, a:b], in0=cur[:, a:b], scalar=2.0 * dw, in1=Rs,
                    op0=AG.mult, op1=AG.add)
                newcur = prev
            if k < K - 1:
                nc.sync.dma_start(
                    ag_in[ci][t0 * P:t1 * P, :].rearrange("(t p) f -> p t f", p=P),
                    newcur[:, a:b].rearrange("p (t f) -> p t f", f=C))

        tsplits = []
        for ci in range(2):
            ts = m[ci]["groups"][0][1]
            for (ta, tb, sg) in m[ci]["groups"]:
                if tb >= padt // 2:
                    ts = tb
                    break
            tsplits.append(ts)

        BIAS = TBL // 2
        for k in range(1, K):
            for ci in range(2):
                mm = m[ci]
                stot, Dt, groups, slot_base = \
                    mm["stot"], mm["Dt"], mm["groups"], mm["slot_base"]
                tsplit = tsplits[ci]
                # k=1 reads the host-uploaded replicated U0 table (no AG).
                ag_out = (u0f_d[ci] if k == 1
                          else ag_outs[ci * (K - 1) + (k - 1)])
                # Signed int16 node indices address the table relative to its
                # midpoint (the Q7 descgen multiply is unsigned*signed).
                table = ag_out[BIAS:TBL, :]

                for (ta, tb, sg) in groups:
                    sbase = slot_base[ta]
                    ng = P * sg
                    G = gp.tile([P, (sgmax + 1) * C], F32, tag="gbuf")
                    g3 = G[:, :sg * C].rearrange("p (s f) -> p s f", f=C)
                    nc.gpsimd.dma_gather(
                        out_ap=g3, in_ap=table,
                        idxs_ap=idxsb[ci][:, sbase * 8:(sbase + sg) * 8],
                        num_idxs=ng, num_idxs_reg=ng, elem_size=C,
                        single_packet=False, queue_num=qrr[0])
                    qrr[0] = (qrr[0] + 1) % 4
                    wb = w2sb[ci][:, sbase:sbase + sg] \
                        .unsqueeze(2).broadcast_to([P, sg, C])
                    nc.vector.tensor_tensor(out=g3, in0=g3, in1=wb, op=AG.mult)
                    for t in range(ta, tb):
                        o = slot_base[t] - sbase
                        msl = G[:, o * C:(o + Dt[t]) * C] \
                            .rearrange("p (u f) -> p f u", f=C)
                        nc.vector.tensor_reduce(
                            out=R[:, t * C:(t + 1) * C], in_=msl,
                            axis=mybir.AxisListType.X, op=AG.add)
                    if tb == tsplit:
                        chain(ci, k, 0, tsplit)
                        if k == K - 1:
                            # Final step only: issue the first half's output
                            # matmuls early so they hide under the remaining
                            # gathers instead of trailing the whole kernel.
                            matmul_acc(ci, k, state[ci]["prev"], first=False,
                                       t0=0, t1=tsplit)

                chain(ci, k, tsplit, padt)
                if k < K - 1:
                    do_allgather(ci, k + 1)
                st = state[ci]
                if k > 1:
                    st["prev"], st["cur"] = st["cur"], st["prev"]
                matmul_acc(ci, k, st["cur"], first=False,
                           t0=tsplit if k == K - 1 else 0)

        for ci in range(2):
            nc.sync.dma_start(nm(out_d[ci]), sb3(accs[ci]))

    nc.compile()
    return nc


# -------------------------------------------------------------------- run
def make_in_maps(meta, W1, b1, W2, b2):
    C = meta["C"]
    wmat = np.zeros((C, 2 * K * C), np.float32)
    for ci, W in enumerate((W1, W2)):
        W = np.asarray(W, np.float32)
        for k in range(K):
            wmat[:, (ci * K + k) * C:(ci * K + k + 1) * C] = W[k]
    brep = np.zeros((P, 2 * C), np.float32)
    brep[:, :C] = np.asarray(b1, np.float32)
    brep[:, C:] = np.asarray(b2, np.float32)
    maps = []
    for c in range(NCORES):
        maps.append({
            "xs0": meta["xs1"][c], "xs1": meta["xs2"][c],
            "u0f0": meta["u0f1"], "u0f1": meta["u0f2"],
            "idx0": meta["m1"]["idx"][c], "idx1": meta["m2"]["idx"][c],
            "w20": meta["m1"]["w2"][c], "w21": meta["m2"]["w2"][c],
            "wmat": wmat, "brep": brep,
        })
    return maps


def assemble(meta, results):
    N, C = meta["N"], meta["C"]
    out = np.empty((N, 2 * C), np.float32)
    for ci, key in enumerate(("out0", "out1")):
        full = np.concatenate([results[c][key] for c in range(NCORES)], 0)
        out[:, ci * C:(ci + 1) * C] = full[meta[f"m{ci + 1}"]["iperm"]]
    return out


def _install_profile_hook():
    import sys, types
    import antenv
    if "antenv.axon_hooks" in sys.modules:
        return
    mod = types.ModuleType("antenv.axon_hooks")
    mod._hook = None
    mod.set_axon_ntff_profile_hook = lambda h: setattr(mod, "_hook", h)
    mod.get_axon_ntff_profile_hook = lambda: mod._hook
    sys.modules["antenv.axon_hooks"] = mod
    antenv.axon_hooks = mod
    from trn_agent_boot.trn_boot import _ntff_profile_via_ctypes
    mod._hook = _ntff_profile_via_ctypes('/opt/axon/libaxon_pjrt.so')


def kernel(x, edge_index, edge_weight, W1, b1, W2, b2):
    global LAST_EXEC_NS, LAST_RESULTS
    x = np.asarray(x)
    meta = prep(x, np.asarray(edge_index), np.asarray(edge_weight))
    nc = build(meta)
    in_maps = make_in_maps(meta, W1, b1, W2, b2)
    trace = TRACE
    if trace:
        try:
            _install_profile_hook()
        except Exception:
            trace = False
    res = bass_utils.run_bass_kernel_spmd(nc, in_maps,
                                          core_ids=list(range(NCORES)),
                                          trace=trace)
    LAST_EXEC_NS = res.exec_time_ns
    LAST_RESULTS = res
    return assemble(meta, res.results)

